# revision 60
# baseline (speedup 1.0000x reference)
"""Trainium2 Bass kernel for MultiHeadAttention with RoPE (cross-attention).

Reference computation (B=4, C=512, T=S=2048, H=8 heads, dc=64):
    q = Wq x + bq ; k = Wk c + bk ; v = Wv c + bv        (1x1 convs)
    q,k <- RoPE(q,k)
    out = softmax(q k^T / 8) v                            (per head)
    y = Wo out + bo

Sharding: 8 cores = (batch b in 0..3) x (T-half j in 0..1).  Core (b,j)
computes the full output slice y[b, :, j*1024:(j+1)*1024] and needs only
x[b,:,tslice] plus all of c[b] (k/v recomputed on both cores of a batch).
No collectives; the host reassembles the 8 disjoint output slices.

Device-side structure:
  * All matmuls run fp16 x fp16 -> fp32 PSUM.  fp16 matmuls register as PE
    activity for the HAM clock gate (fp32r does not), so the clock stays at
    2.4 GHz without keep-alive tricks, and FWL halves LDWEIGHTS time.
  * 1/(sqrt(dc)*16) folded into Wq host-side: the scores PSUM holds s/16.
    exp(s) is computed as exp(16*u): the scalar engine uses activation
    scale=16; the vector engine uses a custom DVE op (cubic minimax poly in
    u then ^16 via squarings) so the 16.8M-element exp load is split across
    both engines instead of serializing on ACT.
  * RoPE(q) = q*cos + (R q)*sin with R the fixed rotate-half matrix; R q is
    one extra 128-contraction matmul on the PE against the fp16 cast of the
    projection (cheaper than a second full projection stream: half the
    projection matmuls and weight DMA).
  * Biases enter via an augmented contraction row when nonzero; with
    all-zero biases those chunks are skipped (except V, see below).
  * scores are computed TRANSPOSED ([s, t] layout) so that exp(scores) can
    be consumed directly by the AV matmul without any transpose.
  * V is projected in transposed layout [s, o] with one extra column per
    head forced to 1.0 (via a 1-row "ones" stationary against the wvT bias
    row); the AV matmul then yields the softmax denominator for free.
  * V projection emits 2x260-wide matmuls per (s-chunk, k-chunk) instead of
    512+8: the 8-wide tail matmuls of the old layout read as PE-idle to the
    HAM monitor and throttled the clock mid-kernel.
  * softmax denominator: reciprocal_approx_fast (DVE) + gpsimd
    partition_broadcast, then one DVE multiply per head/t-block.
  * Consecutive matmuls share a stationary operand (two moving blocks per
    weight load) to halve LDWEIGHTS traffic.
"""

import math
from contextlib import ExitStack

import numpy as np

import concourse.bass as bass
import concourse.tile as tile
from concourse import bacc, mybir
from concourse import bass_utils

F32 = mybir.dt.float32
FP16 = mybir.dt.float16
AF = mybir.ActivationFunctionType

# Problem constants (hardcoded per contract)
B, C, T = 4, 512, 2048
H, DC = 8, 64
THETA = 10000.0
NCORES = 8
TH = T // 2            # 1024: per-core t slice
P = 128                # partitions
NT = 512               # matmul free-dim tile
KC = C // P            # 4 contraction chunks over channels
OC = C // P            # 4 output-channel chunks
SCN = T // P           # 16 s chunks
VW = H * (DC + 1)      # 520: vT width (64 chans + 1 ones col per head)
VH = VW // 2           # 260: half-width for one-bank psum tiles
VWP = VW + P - (DC + 1)  # 583: padded so vt[:, h*65 : h*65+128] stays in-bounds

ESC = 16.0             # extra score downscale; exp applied as exp(ESC * u)

# Minimax cubic p(u) ~= e^u on |u| <= 0.46 (Remez, relative error 2.3e-4).
# The custom DVE op computes p(u)^4; two stock fp16 squarings finish ^16.
EXPC = (0.999784237261267, 1.000506880594075, 0.5086882983844545, 0.16491170234078503)

# which score chunks go to the DVE exp (rest use ACT).  Spread so the last
# DVE chain (~2.4us) completes before its deferred AV matmul comes up.
EXP_DVE_SCS = (2, 5, 8, 11)


def _is_dve_tile(h, sc):
    return sc in EXP_DVE_SCS


# --------------------------------------------------------------------------
# custom DVE op: EXP4 = p(u)^4, p = a0 + u(c0 + u(c1 + u c2))
# --------------------------------------------------------------------------

_EXP4 = None
_SQSQ = None


def _register(name, spec, rd1):
    from concourse.dve_spec import lower
    from concourse import dve_ops as DO
    from concourse.dve_uop import DveOpSpec

    if name not in DO._SUB_OPCODE_FOR_NAME:
        DO._SUB_OPCODE_FOR_NAME[name] = max(DO._SUB_OPCODE_FOR_NAME.values()) + 1
        assert DO._SUB_OPCODE_FOR_NAME[name] < 0x20
    shas = {}
    for ver in ("v3", "v4"):
        try:
            s = DveOpSpec(name=name, opcode=DO.get_dve_sub_opcode(name),
                          uops=lower(spec, ver=ver), rd1_en=rd1)
            shas[ver] = s.sha(ver)
        except Exception:
            pass
    op = DO.DveOp(name, spec, subdim=False, uops_sha=shas)
    if all(o.name != name for o in DO.OPS):
        DO.OPS.append(op)
    return op


def _get_exp4():
    global _EXP4
    if _EXP4 is not None:
        return _EXP4
    from concourse.dve_spec import (
        Spec, Src0, C0, C1, C2, C3, sq, _spill_c3_to_src1,
    )

    def _ref(in0, in1, s0, s1, imm2):
        a0 = np.asarray(in1, np.float32).reshape(in0.shape[0], 1)
        u = np.asarray(in0, np.float32)
        p = a0 + u * (s0 + u * (s1 + u * imm2))
        return ((p * p) * (p * p)).astype(np.float32)

    body = sq(sq(_spill_c3_to_src1(C3 + Src0 * (C0 + Src0 * (C1 + Src0 * C2)))))
    _EXP4 = _register("EXP4_MHA_ANT", Spec(body=body, reference=_ref), True)
    return _EXP4


def _get_sqsq():
    global _SQSQ
    if _SQSQ is not None:
        return _SQSQ
    from concourse.dve_spec import Spec, Src0, sq

    def _ref(in0, in1, s0, s1, imm2):
        x = np.asarray(in0, np.float32)
        return ((x * x) * (x * x)).astype(np.float32)

    _SQSQ = _register("SQSQ_MHA_ANT", Spec(body=sq(sq(Src0)), reference=_ref),
                      False)
    return _SQSQ


# --------------------------------------------------------------------------
# device program
# --------------------------------------------------------------------------

def _emit(tc, d, y, use_bias):
    nc = tc.nc
    NK = KC + 1 if use_bias else KC  # contraction chunks incl. optional bias
    exp4 = _get_exp4()
    sqsq = _get_sqsq()

    with ExitStack() as ctx:
        # ---------- persistent pools (live to end of kernel), LIFO-ordered
        p_ms = ctx.enter_context(tc.tile_pool(name="p_ms", bufs=1))
        p_q = ctx.enter_context(tc.tile_pool(name="p_q", bufs=1))
        p_k = ctx.enter_context(tc.tile_pool(name="p_k", bufs=1))
        p_vt = ctx.enter_context(tc.tile_pool(name="p_vt", bufs=1))
        p_c = ctx.enter_context(tc.tile_pool(name="p_c", bufs=1))
        p_wv = ctx.enter_context(tc.tile_pool(name="p_wv", bufs=1))
        p_wo = ctx.enter_context(tc.tile_pool(name="p_wo", bufs=1))
        p_att = ctx.enter_context(tc.tile_pool(name="p_att", bufs=1))
        pwq = ctx.enter_context(tc.tile_pool(name="pwq", bufs=1))
        px = ctx.enter_context(tc.tile_pool(name="px", bufs=1))
        # attention-phase SBUF pools are persistent: a pool opened after
        # earlier pools close would reuse their addresses and inherit
        # spurious cross-engine WAR waits on first use
        ppt = ctx.enter_context(tc.tile_pool(name="ppt", bufs=4))
        pp48 = ctx.enter_context(tc.tile_pool(name="pp48", bufs=4))
        pden = ctx.enter_context(tc.tile_pool(name="pden", bufs=2))

        # PE warm-up operand: the FIRST DVE op emitted, so the warm-up
        # matmuls' semaphore wait clears immediately (Tile waits on the
        # engine's op counter — anything emitted before this on DVE would
        # delay the warm-up past the point of it).
        wz = p_ms.tile([P, NT], FP16, name="wz", tag="wz")
        nc.vector.memset(wz[:], 0.0)

        # q is stored per head in [128, TH] tiles: head h's rotated q sits
        # at rows (h%2)*64..+64, the other 64 rows are zero.  The scores
        # matmul then runs with a full 128x128 stationary (k, two heads
        # packed) against the masked q — the zero rows cancel the foreign
        # head.  Init memsets go to the (idle) GpSimd engine so they don't
        # queue ahead of real DVE work; none is consumed before ~40us.
        q_sb = [p_q.tile([P, TH], FP16, name=f"q{i}", tag=f"q{i}")
                for i in range(H)]
        for h in range(H):
            zr = slice(DC, P) if h % 2 == 0 else slice(0, DC)
            nc.gpsimd.memset(q_sb[h][zr, :], 0.0)
        k_sb = [p_k.tile([P, T], FP16, name=f"k{i}", tag=f"k{i}")
                for i in range(OC)]
        vt_sb = [p_vt.tile([P, VWP], FP16, name=f"vt{i}", tag=f"vt{i}")
                 for i in range(SCN)]
        for i in range(SCN):
            nc.gpsimd.memset(vt_sb[i][:, VW:VWP], 0.0)
        att_sb = [p_att.tile([P, TH], FP16, name=f"att{i}", tag=f"att{i}")
                  for i in range(OC)]
        # [P,1] broadcast of the cubic's a0 for the custom DVE op (C3 spill)
        a0bc = p_ms.tile([P, 1], F32, name="a0bc", tag="a0bc")
        nc.gpsimd.memset(a0bc[:], EXPC[0])
        # all-ones stationary row for bias-row / ones-column contractions
        ones1 = p_ms.tile([1, T], FP16, name="ones1", tag="ones1")
        nc.gpsimd.memset(ones1[:], 1.0)

        # ---------- input DMAs, emitted in consumption order
        def load_w(pool, dname, width, brow=None):
            chunks = []
            for i in range(KC):
                t_ = pool.tile([P, width], FP16, name=f"{dname}{i}",
                               tag=f"{dname}{i}")
                nc.sync.dma_start(t_[:], d[dname][i * P:(i + 1) * P, :])
                chunks.append(t_)
            if use_bias and brow is not None:
                nc.sync.dma_start(brow[0:1, :], d[dname][C:C + 1, :])
                chunks.append(brow)  # chunks[KC] is the bias row
            return chunks

        with tc.tile_pool(name="pps", space="PSUM", bufs=8) as pps:
          if True:
            # Q-phase inputs first, chunk-interleaved so the first matmul's
            # operands land before the bulk weights
            bq1 = pwq.tile([1, C], FP16, name="bq1", tag="bq1")
            rot_t = p_ms.tile([P, P], FP16, name="rot_t", tag="rot_t")
            nc.sync.dma_start(rot_t[:], d["rotT"][:])
            wq_ch, x_ch = [], []
            for i in range(KC):
                wt = pwq.tile([P, C], FP16, name=f"wqT{i}", tag=f"wqT{i}")
                nc.sync.dma_start(wt[:], d["wqT"][i * P:(i + 1) * P, :])
                wq_ch.append(wt)
                t_ = px.tile([P, TH], FP16, name=f"x{i}", tag=f"x{i}")
                nc.sync.dma_start(t_[:], d["xh"][i * P:(i + 1) * P, :])
                x_ch.append(t_)
            if use_bias:
                nc.sync.dma_start(bq1[0:1, :], d["wqT"][C:C + 1, :])
                wq_ch.append(bq1)
                xb = px.tile([1, TH], FP16, name="xb", tag="xb")
                nc.sync.dma_start(xb[:], d["xh"][C:C + 1, :])
                x_ch.append(xb)
            cosq = px.tile([P, TH], FP16, name="cosq", tag="cosq")
            nc.sync.dma_start(cosq[:], d["cosq"][:])
            sinq = px.tile([P, TH], FP16, name="sinq", tag="sinq")
            nc.sync.dma_start(sinq[:], d["sinq"][:])
            # wk behind the Q inputs but ahead of the c/cos/wv/wo bulk: the
            # K phase starts ~27us in and must not wait on 4MB of queue
            wk_ch = []
            for i in range(KC):
                wt = pwq.tile([P, C], FP16, name=f"wkT{i}", tag=f"wkT{i}")
                nc.sync.dma_start(wt[:], d["wkT"][i * P:(i + 1) * P, :])
                wk_ch.append(wt)

            # PE warm-up during the DMA lead-in: matmuls on a zeroed tile
            # count as PE activity, so the HAM clock gate reaches K=8/8
            # before the first real matmul instead of ~3.4us into Q.
            pz = pps.tile([P, NT], F32, name="pz", tag="ps")
            for i in range(3):
                nc.tensor.matmul(pz[:], wz[:, 0:P], wz[:],
                                 start=(i == 0), stop=(i == 2))

            # persistent-pool inputs: emitted early so their DMAs queue
            # behind the q-stage inputs and stream during q compute
            c_ch = []
            for i in range(KC):
                t_ = p_c.tile([P, T], FP16, name=f"c{i}", tag=f"c{i}")
                nc.sync.dma_start(t_[:], d["ch"][i * P:(i + 1) * P, :])
                c_ch.append(t_)
            wv_ch = []
            for i in range(KC + 1):
                rows = slice(i * P, (i + 1) * P) if i < KC else slice(C, C + 1)
                t_ = p_wv.tile([P if i < KC else 1, VW], FP16,
                               name=f"wv{i}", tag=f"wv{i}")
                nc.sync.dma_start(t_[:], d["wvT"][rows, :])
                wv_ch.append(t_)
            wo_ch = []
            for i in range(KC):
                t_ = p_wo.tile([P, C], FP16, name=f"wo{i}", tag=f"wo{i}")
                nc.sync.dma_start(t_[:], d["woT"][i * P:(i + 1) * P, :])
                wo_ch.append(t_)
            bo4 = p_ms.tile([P, OC], F32, name="bo4", tag="bo4")
            nc.sync.dma_start(bo4[:], d["bo4"][:])

          # ---- Q projection + RoPE (projection, then rot = R q on the PE) --
          # The rot matmuls + RoPE tail for block m are emitted inside block
          # m+1's projection loop so the PE never waits on the psum->fp16
          # cast round-trip.
          if True:
            def flush_q_rot(m, qcs):
                for tb in range(2):
                    fs = slice(tb * NT, (tb + 1) * NT)
                    rp = pps.tile([P, NT], F32, name="rp", tag="ps")
                    nc.tensor.matmul(rp[:], rot_t[:], qcs[tb][:],
                                     start=True, stop=True)
                    t1 = px.tile([P, NT], FP16, name="rt1", tag="rt1", bufs=2)
                    nc.vector.tensor_mul(t1[:], qcs[tb][:], cosq[:, fs])
                    t2 = px.tile([P, NT], FP16, name="rt2", tag="rt2", bufs=2)
                    nc.vector.tensor_mul(t2[:], rp[:], sinq[:, fs])
                    nc.vector.tensor_add(q_sb[2 * m][0:DC, fs],
                                         t1[0:DC, :], t2[0:DC, :])
                    nc.vector.tensor_add(q_sb[2 * m + 1][DC:P, fs],
                                         t1[DC:P, :], t2[DC:P, :])

            carry_q = None
            for m in range(OC):
              ocs = slice(m * P, (m + 1) * P)
              psq = [pps.tile([P, NT], F32, name="psq", tag="ps")
                     for _ in range(2)]
              for kc in range(NK):
                st, spq = (kc == 0), (kc == NK - 1)
                for tb in range(2):  # same stationary, two t-blocks
                    fs = slice(tb * NT, (tb + 1) * NT)
                    nc.tensor.matmul(psq[tb][:], wq_ch[kc][:, ocs],
                                     x_ch[kc][:, fs], start=st, stop=spq)
                if kc == 1 and carry_q is not None:
                    flush_q_rot(*carry_q)
                    carry_q = None
              qcs = []
              for tb in range(2):
                qc = px.tile([P, NT], FP16, name="qc1", tag="qc1", bufs=4)
                nc.scalar.activation(qc[:], psq[tb][:], AF.Copy)
                qcs.append(qc)
              carry_q = (m, qcs)
            flush_q_rot(*carry_q)
          # ---- K projection + RoPE (projection, then rot = R k on the PE) --
          with tc.tile_pool(name="pwk", bufs=1) as pwk, \
               tc.tile_pool(name="ptk", bufs=1) as ptk:
            if use_bias:
                bk1 = pwk.tile([1, C], FP16, name="bk1", tag="bk1")
                nc.sync.dma_start(bk1[0:1, :], d["wkT"][C:C + 1, :])
                wk_ch = wk_ch + [bk1]
            cosk = ptk.tile([P, T], FP16, name="cosk", tag="cosk")
            nc.sync.dma_start(cosk[:], d["cosk"][:])
            sink = ptk.tile([P, T], FP16, name="sink", tag="sink")
            nc.sync.dma_start(sink[:], d["sink"][:])

            def flush_k_rot(oc, sbp, kcs):
                for i in range(2):
                    fs = slice((2 * sbp + i) * NT, (2 * sbp + i + 1) * NT)
                    rp = pps.tile([P, NT], F32, name="rpk", tag="ps")
                    nc.tensor.matmul(rp[:], rot_t[:], kcs[i][:],
                                     start=True, stop=True)
                    t1 = ptk.tile([P, NT], FP16, name="kt1", tag="kt1",
                                  bufs=2)
                    nc.vector.tensor_mul(t1[:], kcs[i][:], cosk[:, fs])
                    t2 = ptk.tile([P, NT], FP16, name="kt2", tag="kt2",
                                  bufs=2)
                    nc.vector.tensor_mul(t2[:], rp[:], sink[:, fs])
                    nc.vector.tensor_add(k_sb[oc][:, fs], t1[:], t2[:])

            carry_k = None
            for oc in range(OC):
                ocs = slice(oc * P, (oc + 1) * P)
                for sbp in range(2):  # s-block pairs
                    psk = [pps.tile([P, NT], F32, name="psk", tag="ps")
                           for _ in range(2)]
                    for kc in range(NK):
                        st, sp = (kc == 0), (kc == NK - 1)
                        cap = c_ch[kc] if kc < KC else ones1[0:1, :]
                        for i in range(2):
                            fs = slice((2 * sbp + i) * NT,
                                       (2 * sbp + i + 1) * NT)
                            nc.tensor.matmul(psk[i][:], wk_ch[kc][:, ocs],
                                             cap[:, fs], start=st, stop=sp)
                        if kc == 1 and carry_k is not None:
                            flush_k_rot(*carry_k)
                            carry_k = None
                    kcs = []
                    for i in range(2):
                        kcv = ptk.tile([P, NT], FP16, name="kc1", tag="kc1",
                                       bufs=4)
                        nc.scalar.activation(kcv[:], psk[i][:], AF.Copy)
                        kcs.append(kcv)
                    carry_k = (oc, sbp, kcs)
            flush_k_rot(*carry_k)

          # ---- V projection (transposed layout [s, o], 2x260-wide psum) ----
          # Shares the Q/K psum pool (a separate pool would add a scheduling
          # barrier at the K->V transition).  The wvT bias row (against the
          # all-ones stationary row) supplies the per-head ones columns (and
          # biases, when present).
          if True:
            for sc in range(SCN):
                scs = slice(sc * P, (sc + 1) * P)
                pv = [pps.tile([P, VH], F32, name=f"pv{i}", tag="ps")
                      for i in range(2)]
                for kc in range(KC + 1):
                    st, sp = (kc == 0), (kc == KC)
                    cap = (c_ch[kc][:, scs] if kc < KC
                           else ones1[0:1, scs])
                    for i in range(2):
                        cols = slice(i * VH, (i + 1) * VH)
                        nc.tensor.matmul(pv[i][:], cap,
                                         wv_ch[kc][:, cols], start=st, stop=sp)
                for i in range(2):
                    cols = slice(i * VH, (i + 1) * VH)
                    # on the scalar engine: the vector engine must be clear
                    # of V-phase work when the attention exp chains start
                    nc.scalar.activation(vt_sb[sc][:, cols], pv[i][:],
                                         AF.Copy)

        # ---- attention: scoresT -> exp -> AV (+denominator) -> normalize --
        with tc.tile_pool(name="ppsc", space="PSUM", bufs=2) as ppsc, \
             tc.tile_pool(name="ppav", space="PSUM", bufs=4) as ppav:
            # Denominator chains are software-pipelined one head back: head
            # h's reciprocal/broadcast/normalize is emitted in the middle of
            # head h+1's loop so its engine-FIFO slots never sit between the
            # PE's AV matmuls and the next head's score-psum reuse waits.
            def emit_den(av, m, rows):
                for tb in range(2):
                    fs = slice(tb * NT, (tb + 1) * NT)
                    # custom DVE ops misread PSUM rows at partition offset
                    # 64 — stage the denominator row to SBUF first
                    dr = pden.tile([1, NT], F32, name="dr", tag="dr",
                                   bufs=4)
                    nc.vector.tensor_copy(dr[0:1, :], av[tb][DC:DC + 1, :])
                    dn = pden.tile([1, NT], F32, name="dn", tag="dn",
                                   bufs=4)
                    nc.vector.reciprocal_approx_fast(dn[0:1, :], dr[0:1, :])
                    bcs = pden.tile([DC, NT], F32, name="bcs", tag="bcs",
                                    bufs=2)
                    nc.gpsimd.partition_broadcast(bcs[:], dn[0:1, :])
                    nc.vector.tensor_mul(att_sb[m][rows, fs],
                                         av[tb][0:DC, :], bcs[:])

            carry = None
            for m in range(OC):          # head pair (heads 2m, 2m+1)
                for hp in range(2):
                    h = 2 * m + hp
                    rows = slice(hp * DC, (hp + 1) * DC)
                    hs = slice(h * (DC + 1), h * (DC + 1) + P)
                    av = [ppav.tile([P, NT], F32, name="av", tag="av")
                          for _ in range(2)]
                    # AV accumulation into PSUM is order-independent, so AV
                    # matmuls are deferred: 1 chunk for ACT-exp'd tiles (hides
                    # ACT latency), to head-end for DVE-exp'd tiles (the DVE
                    # chain is ~2.4us; issuing its AV in sc order would stall
                    # the PE's in-order queue and oscillate the HAM clock).
                    pend = []   # (pt tile, sc) not yet fed to AV
                    dve_pend = []
                    n_issued = 0

                    def issue_av(pt, sc, first, last):
                        for tb in range(2):  # same vT stationary (128 wide)
                            nc.tensor.matmul(av[tb][:], vt_sb[sc][:, hs],
                                             pt[:, tb * NT:(tb + 1) * NT],
                                             start=first, stop=last)

                    for sc in range(SCN):
                        scs = slice(sc * P, (sc + 1) * P)
                        sp = ppsc.tile([P, 2 * NT], F32, name="sp", tag="sp")
                        for tb in range(2):  # same k-head stationary
                            fs = slice(tb * NT, (tb + 1) * NT)
                            nc.tensor.matmul(sp[:, tb * NT:(tb + 1) * NT],
                                             k_sb[m][:, scs], q_sb[h][:, fs],
                                             start=True, stop=True)
                        pt = ppt.tile([P, 2 * NT], FP16, name="pt", tag="pt",
                                      bufs=8)
                        if _is_dve_tile(h, sc):
                            p4 = pp48.tile([P, 2 * NT], FP16, name="p4",
                                           tag="p4", bufs=2)
                            nc.vector._custom_dve(
                                exp4, out=p4[:], in0=sp[:], in1=a0bc[:],
                                s0=EXPC[1], s1=EXPC[2], imm2=EXPC[3])
                            # single fused ^4 op keeps the vector FIFO short
                            # (exp4 completions gate score psum-slot reuse)
                            nc.vector._custom_dve(sqsq, out=pt[:], in0=p4[:])
                            dve_pend.append((pt, sc))
                        else:
                            nc.scalar.activation(pt[:], sp[:], AF.Exp,
                                                 scale=ESC)
                            pend.append((pt, sc))
                        # previous head's denominator work, mid-head
                        if sc == 3 and carry is not None:
                            emit_den(*carry)
                            carry = None
                        # flush one ACT tile, two sc behind (covers the
                        # ~1.1us ACT exp latency without stalling the PE)
                        if len(pend) >= 3:
                            fpt, fsc = pend.pop(0)
                            issue_av(fpt, fsc, n_issued == 0, False)
                            n_issued += 1
                    tail = pend + dve_pend
                    for i, (fpt, fsc) in enumerate(tail):
                        issue_av(fpt, fsc, n_issued == 0,
                                 i == len(tail) - 1)
                        n_issued += 1
                    carry = (av, m, rows)
            emit_den(*carry)

        # ---- output projection + bias ----
        with tc.tile_pool(name="ppy", space="PSUM", bufs=4) as ppy, \
             tc.tile_pool(name="pys", bufs=4) as pys:
            for of in range(OC):
                ofs = slice(of * P, (of + 1) * P)
                yp = [ppy.tile([P, NT], F32, name="yp", tag="yp")
                      for _ in range(2)]
                for oc in range(KC):
                    st, sp = (oc == 0), (oc == KC - 1)
                    for tb in range(2):  # same Wo stationary
                        fs = slice(tb * NT, (tb + 1) * NT)
                        nc.tensor.matmul(yp[tb][:], wo_ch[oc][:, ofs],
                                         att_sb[oc][:, fs], start=st, stop=sp)
                for tb in range(2):
                    fs = slice(tb * NT, (tb + 1) * NT)
                    ys = pys.tile([P, NT], FP16, name="ys", tag="ys")
                    nc.vector.tensor_scalar_add(ys[:], yp[tb][:],
                                                bo4[:, of:of + 1])
                    nc.sync.dma_start(y[ofs, fs], ys[:])


def build_program(use_bias=False):
    nc = bacc.Bacc("TRN2", target_bir_lowering=False, debug=False,
                   num_devices=NCORES)
    d = {}
    CE = C + 1 if use_bias else C

    def din(name, shape, dt=FP16):
        d[name] = nc.dram_tensor(name, shape, dt, kind="ExternalInput").ap()

    din("xh", [CE, TH])
    din("ch", [C, T])
    din("wqT", [CE, C])
    din("wkT", [CE, C])
    din("rotT", [P, P])
    din("wvT", [C + 1, VW])
    din("woT", [C, C])
    din("bo4", [P, OC], F32)
    din("cosq", [P, TH])
    din("sinq", [P, TH])
    din("cosk", [P, T])
    din("sink", [P, T])
    y = nc.dram_tensor("y", [C, TH], FP16, kind="ExternalOutput").ap()

    with tile.TileContext(nc) as tc:
        _emit(tc, d, y, use_bias)
    nc.compile()
    return nc


# --------------------------------------------------------------------------
# host-side input prep / output assembly
# --------------------------------------------------------------------------

def _rot_matrix():
    """R such that (R q)[i] matches reference rotate-half per 64-chan head."""
    R = np.zeros((C, C), np.float32)
    half = DC // 2
    for h in range(H):
        b0 = h * DC
        for i in range(half):
            R[b0 + i, b0 + half + i] = -1.0
            R[b0 + half + i, b0 + i] = 1.0
    return R


def _rope_tables():
    inv = 1.0 / (THETA ** (np.arange(0, DC, 2, dtype=np.float32) / DC))  # [32]
    f = np.arange(T, dtype=np.float32)[:, None] * inv[None, :]           # [T,32]
    pos = np.concatenate([f, f], axis=-1)                                # [T,64]
    cos_t, sin_t = np.cos(pos), np.sin(pos)                              # [T,64]
    # [128, T]: row r covers head-pair channel r, channel dim = r % 64
    cos_tab = np.ascontiguousarray(np.tile(cos_t.T, (2, 1)), np.float16)
    sin_tab = np.ascontiguousarray(np.tile(sin_t.T, (2, 1)), np.float16)
    return cos_tab, sin_tab


def make_in_maps(x, c, Wq, bq, Wk, bk, Wv, bv, Wo, bo, use_bias):
    scale = 1.0 / (math.sqrt(DC) * ESC)
    Wq_s, bq_s = Wq * scale, bq * scale

    def aug(Wt, bias):
        if not use_bias:
            return np.ascontiguousarray(Wt, np.float16)
        return np.ascontiguousarray(
            np.concatenate([Wt, bias[None, :]], axis=0), np.float16)

    wqT = aug(Wq_s.T, bq_s)
    wkT = aug(Wk.T, bk)
    # rotate-half matrix for a 2-head 128-row block, pre-transposed for
    # the stationary operand (rot = R q via lhsT = R.T)
    rotT = np.ascontiguousarray(_rot_matrix()[:P, :P].T, np.float16)

    # V^T always augmented: bias row carries per-head ones cols (h*65+64)
    wvT = np.zeros((C + 1, VW), np.float32)
    for h in range(H):
        wvT[:C, h * (DC + 1):h * (DC + 1) + DC] = Wv[h * DC:(h + 1) * DC, :].T
        wvT[C, h * (DC + 1):h * (DC + 1) + DC] = bv[h * DC:(h + 1) * DC]
        wvT[C, h * (DC + 1) + DC] = 1.0  # ones column via the bias row
    wvT = wvT.astype(np.float16)

    woT = np.ascontiguousarray(Wo.T, np.float16)
    bo4 = np.ascontiguousarray(bo.reshape(OC, P).T, np.float32)

    cos_tab, sin_tab = _rope_tables()

    ones_t = np.ones((1, TH), np.float16)
    in_maps = []
    for core in range(NCORES):
        b, j = core // 2, core % 2
        ts = slice(j * TH, (j + 1) * TH)
        if use_bias:
            xh = np.concatenate([x[b][:, ts].astype(np.float16), ones_t],
                                axis=0)
        else:
            xh = x[b][:, ts].astype(np.float16)
        in_maps.append({
            "xh": np.ascontiguousarray(xh, np.float16),
            "ch": np.ascontiguousarray(c[b], np.float16),
            "wqT": wqT, "wkT": wkT, "rotT": rotT,
            "wvT": wvT, "woT": woT, "bo4": bo4,
            "cosq": np.ascontiguousarray(cos_tab[:, ts]),
            "sinq": np.ascontiguousarray(sin_tab[:, ts]),
            "cosk": cos_tab,
            "sink": sin_tab,
        })
    return in_maps


def assemble(results):
    Y = np.empty((B, C, T), np.float32)
    for core in range(NCORES):
        b, j = core // 2, core % 2
        Y[b, :, j * TH:(j + 1) * TH] = results[core]["y"]
    return Y


_CACHE = {}


def _get_program(use_bias):
    if use_bias not in _CACHE:
        _CACHE[use_bias] = build_program(use_bias)
    return _CACHE[use_bias]


def run(trace=False, **inputs):
    use_bias = any(
        np.any(np.asarray(inputs[k])) for k in ("bq", "bk", "bv"))
    nc = _get_program(use_bias)
    in_maps = make_in_maps(use_bias=use_bias, **inputs)
    res = bass_utils.run_bass_kernel_spmd(
        nc, in_maps, core_ids=list(range(NCORES)), trace=trace)
    return assemble(res.results), res


def kernel(**inputs):
    out, _ = run(trace=False, **inputs)
    return out


# revision 61
# speedup vs baseline: 1.1913x; 1.1913x over previous
"""Trainium2 Bass kernel for MultiHeadAttention with RoPE (cross-attention).

Reference computation (B=4, C=512, T=S=2048, H=8 heads, dc=64):
    q = Wq x + bq ; k = Wk c + bk ; v = Wv c + bv        (1x1 convs)
    q,k <- RoPE(q,k)
    out = softmax(q k^T / 8) v                            (per head)
    y = Wo out + bo

Sharding: 8 cores = (batch b in 0..3) x (T-half j in 0..1).  Core (b,j)
computes the full output slice y[b, :, j*1024:(j+1)*1024] and needs only
x[b,:,tslice] plus all of c[b] (k/v recomputed on both cores of a batch).
No collectives; the host reassembles the 8 disjoint output slices.

Device-side structure:
  * All matmuls run fp16 x fp16 -> fp32 PSUM.  fp16 matmuls register as PE
    activity for the HAM clock gate (fp32r does not), so the clock stays at
    2.4 GHz without keep-alive tricks, and FWL halves LDWEIGHTS time.
  * 1/(sqrt(dc)*16) folded into Wq host-side: the scores PSUM holds s/16.
    exp(s) is computed as exp(16*u): the scalar engine uses activation
    scale=16; the vector engine uses a custom DVE op (cubic minimax poly in
    u then ^16 via squarings) so the 16.8M-element exp load is split across
    both engines instead of serializing on ACT.
  * RoPE(q) = q*cos + (R q)*sin with R the fixed rotate-half matrix; R q is
    one extra 128-contraction matmul on the PE against the fp16 cast of the
    projection (cheaper than a second full projection stream: half the
    projection matmuls and weight DMA).
  * Biases enter via an augmented contraction row when nonzero; with
    all-zero biases those chunks are skipped (except V, see below).
  * scores are computed TRANSPOSED ([s, t] layout) so that exp(scores) can
    be consumed directly by the AV matmul without any transpose.
  * V is projected in transposed layout [s, o] with one extra column per
    head forced to 1.0 (via a 1-row "ones" stationary against the wvT bias
    row); the AV matmul then yields the softmax denominator for free.
  * V projection emits 2x260-wide matmuls per (s-chunk, k-chunk) instead of
    512+8: the 8-wide tail matmuls of the old layout read as PE-idle to the
    HAM monitor and throttled the clock mid-kernel.
  * softmax denominator: reciprocal_approx_fast (DVE) + gpsimd
    partition_broadcast, then one DVE multiply per head/t-block.
  * Consecutive matmuls share a stationary operand (two moving blocks per
    weight load) to halve LDWEIGHTS traffic.
"""

import math
from contextlib import ExitStack

import numpy as np

import concourse.bass as bass
import concourse.tile as tile
from concourse import bacc, mybir
from concourse import bass_utils

F32 = mybir.dt.float32
FP16 = mybir.dt.float16
AF = mybir.ActivationFunctionType

# Problem constants (hardcoded per contract)
B, C, T = 4, 512, 2048
H, DC = 8, 64
THETA = 10000.0
NCORES = 8
TH = T // 2            # 1024: per-core t slice
P = 128                # partitions
NT = 512               # matmul free-dim tile
KC = C // P            # 4 contraction chunks over channels
OC = C // P            # 4 output-channel chunks
SCN = T // P           # 16 s chunks
VW = H * (DC + 1)      # 520: vT width (64 chans + 1 ones col per head)
VH = VW // 2           # 260: half-width for one-bank psum tiles
VWP = VW + P - (DC + 1)  # 583: padded so vt[:, h*65 : h*65+128] stays in-bounds

ESC = 16.0             # extra score downscale; exp applied as exp(ESC * u)

# Minimax cubic p(u) ~= e^u on |u| <= 0.46 (Remez, relative error 2.3e-4).
# The custom DVE op computes p(u)^4; two stock fp16 squarings finish ^16.
EXPC = (0.999784237261267, 1.000506880594075, 0.5086882983844545, 0.16491170234078503)

# which score chunks go to the DVE exp (rest use ACT).  Spread so the last
# DVE chain (~2.4us) completes before its deferred AV matmul comes up.
EXP_DVE_SCS = (2, 5, 8, 11)


def _is_dve_tile(h, sc):
    return sc in EXP_DVE_SCS


# --------------------------------------------------------------------------
# custom DVE op: EXP4 = p(u)^4, p = a0 + u(c0 + u(c1 + u c2))
# --------------------------------------------------------------------------

_EXP4 = None
_SQSQ = None


def _register(name, spec, rd1):
    from concourse.dve_spec import lower
    from concourse import dve_ops as DO
    from concourse.dve_uop import DveOpSpec

    if name not in DO._SUB_OPCODE_FOR_NAME:
        DO._SUB_OPCODE_FOR_NAME[name] = max(DO._SUB_OPCODE_FOR_NAME.values()) + 1
        assert DO._SUB_OPCODE_FOR_NAME[name] < 0x20
    shas = {}
    for ver in ("v3", "v4"):
        try:
            s = DveOpSpec(name=name, opcode=DO.get_dve_sub_opcode(name),
                          uops=lower(spec, ver=ver), rd1_en=rd1)
            shas[ver] = s.sha(ver)
        except Exception:
            pass
    op = DO.DveOp(name, spec, subdim=False, uops_sha=shas)
    if all(o.name != name for o in DO.OPS):
        DO.OPS.append(op)
    return op


def _get_exp4():
    global _EXP4
    if _EXP4 is not None:
        return _EXP4
    from concourse.dve_spec import (
        Spec, Src0, C0, C1, C2, C3, sq, _spill_c3_to_src1,
    )

    def _ref(in0, in1, s0, s1, imm2):
        a0 = np.asarray(in1, np.float32).reshape(in0.shape[0], 1)
        u = np.asarray(in0, np.float32)
        p = a0 + u * (s0 + u * (s1 + u * imm2))
        return ((p * p) * (p * p)).astype(np.float32)

    body = sq(sq(_spill_c3_to_src1(C3 + Src0 * (C0 + Src0 * (C1 + Src0 * C2)))))
    _EXP4 = _register("EXP4_MHA_ANT", Spec(body=body, reference=_ref), True)
    return _EXP4


def _get_sqsq():
    global _SQSQ
    if _SQSQ is not None:
        return _SQSQ
    from concourse.dve_spec import Spec, Src0, sq

    def _ref(in0, in1, s0, s1, imm2):
        x = np.asarray(in0, np.float32)
        return ((x * x) * (x * x)).astype(np.float32)

    _SQSQ = _register("SQSQ_MHA_ANT", Spec(body=sq(sq(Src0)), reference=_ref),
                      False)
    return _SQSQ


# --------------------------------------------------------------------------
# device program
# --------------------------------------------------------------------------

def _emit(tc, d, y, use_bias):
    nc = tc.nc
    NK = KC + 1 if use_bias else KC  # contraction chunks incl. optional bias
    exp4 = _get_exp4()
    sqsq = _get_sqsq()

    with ExitStack() as ctx:
        # ---------- persistent pools (live to end of kernel), LIFO-ordered
        p_ms = ctx.enter_context(tc.tile_pool(name="p_ms", bufs=1))
        p_q = ctx.enter_context(tc.tile_pool(name="p_q", bufs=1))
        p_k = ctx.enter_context(tc.tile_pool(name="p_k", bufs=1))
        p_vt = ctx.enter_context(tc.tile_pool(name="p_vt", bufs=1))
        p_c = ctx.enter_context(tc.tile_pool(name="p_c", bufs=1))
        p_wv = ctx.enter_context(tc.tile_pool(name="p_wv", bufs=1))
        p_wo = ctx.enter_context(tc.tile_pool(name="p_wo", bufs=1))
        p_att = ctx.enter_context(tc.tile_pool(name="p_att", bufs=1))
        pwq = ctx.enter_context(tc.tile_pool(name="pwq", bufs=1))
        px = ctx.enter_context(tc.tile_pool(name="px", bufs=1))
        # attention-phase SBUF pools are persistent: a pool opened after
        # earlier pools close would reuse their addresses and inherit
        # spurious cross-engine WAR waits on first use
        ppt = ctx.enter_context(tc.tile_pool(name="ppt", bufs=4))
        pp48 = ctx.enter_context(tc.tile_pool(name="pp48", bufs=4))
        pden = ctx.enter_context(tc.tile_pool(name="pden", bufs=2))

        # PE warm-up operand: the FIRST DVE op emitted, so the warm-up
        # matmuls' semaphore wait clears immediately (Tile waits on the
        # engine's op counter — anything emitted before this on DVE would
        # delay the warm-up past the point of it).
        wz = p_ms.tile([P, NT], FP16, name="wz", tag="wz")
        nc.vector.memset(wz[:], 0.0)

        # q is stored per head in [128, TH] tiles: head h's rotated q sits
        # at rows (h%2)*64..+64, the other 64 rows are zero.  The scores
        # matmul then runs with a full 128x128 stationary (k, two heads
        # packed) against the masked q — the zero rows cancel the foreign
        # head.  Init memsets go to the (idle) GpSimd engine so they don't
        # queue ahead of real DVE work; none is consumed before ~40us.
        q_sb = [p_q.tile([P, TH], FP16, name=f"q{i}", tag=f"q{i}")
                for i in range(H)]
        for h in range(H):
            zr = slice(DC, P) if h % 2 == 0 else slice(0, DC)
            nc.gpsimd.memset(q_sb[h][zr, :], 0.0)
        k_sb = [p_k.tile([P, T], FP16, name=f"k{i}", tag=f"k{i}")
                for i in range(OC)]
        vt_sb = [p_vt.tile([P, VWP], FP16, name=f"vt{i}", tag=f"vt{i}")
                 for i in range(SCN)]
        for i in range(SCN):
            nc.gpsimd.memset(vt_sb[i][:, VW:VWP], 0.0)
        att_sb = [p_att.tile([P, TH], FP16, name=f"att{i}", tag=f"att{i}")
                  for i in range(OC)]
        # [P,1] broadcast of the cubic's a0 for the custom DVE op (C3 spill)
        a0bc = p_ms.tile([P, 1], F32, name="a0bc", tag="a0bc")
        nc.gpsimd.memset(a0bc[:], EXPC[0])
        # all-ones stationary row for bias-row / ones-column contractions
        ones1 = p_ms.tile([1, T], FP16, name="ones1", tag="ones1")
        nc.gpsimd.memset(ones1[:], 1.0)

        # ---------- input DMAs, emitted in consumption order
        def load_w(pool, dname, width, brow=None):
            chunks = []
            for i in range(KC):
                t_ = pool.tile([P, width], FP16, name=f"{dname}{i}",
                               tag=f"{dname}{i}")
                nc.sync.dma_start(t_[:], d[dname][i * P:(i + 1) * P, :])
                chunks.append(t_)
            if use_bias and brow is not None:
                nc.sync.dma_start(brow[0:1, :], d[dname][C:C + 1, :])
                chunks.append(brow)  # chunks[KC] is the bias row
            return chunks

        with tc.tile_pool(name="pps", space="PSUM", bufs=8) as pps:
          if True:
            # Q-phase inputs first, chunk-interleaved so the first matmul's
            # operands land before the bulk weights
            bq1 = pwq.tile([1, C], FP16, name="bq1", tag="bq1")
            rot_t = p_ms.tile([P, P], FP16, name="rot_t", tag="rot_t")
            nc.sync.dma_start(rot_t[:], d["rotT"][:])
            wq_ch, x_ch = [], []
            for i in range(KC):
                wt = pwq.tile([P, C], FP16, name=f"wqT{i}", tag=f"wqT{i}")
                nc.sync.dma_start(wt[:], d["wqT"][i * P:(i + 1) * P, :])
                wq_ch.append(wt)
                t_ = px.tile([P, TH], FP16, name=f"x{i}", tag=f"x{i}")
                nc.sync.dma_start(t_[:], d["xh"][i * P:(i + 1) * P, :])
                x_ch.append(t_)
            if use_bias:
                nc.sync.dma_start(bq1[0:1, :], d["wqT"][C:C + 1, :])
                wq_ch.append(bq1)
                xb = px.tile([1, TH], FP16, name="xb", tag="xb")
                nc.sync.dma_start(xb[:], d["xh"][C:C + 1, :])
                x_ch.append(xb)
            cosq = px.tile([P, TH], FP16, name="cosq", tag="cosq")
            nc.sync.dma_start(cosq[:], d["cosq"][:])
            sinq = px.tile([P, TH], FP16, name="sinq", tag="sinq")
            nc.sync.dma_start(sinq[:], d["sinq"][:])
            # wk behind the Q inputs but ahead of the c/cos/wv/wo bulk: the
            # K phase starts ~27us in and must not wait on 4MB of queue
            wk_ch = []
            for i in range(KC):
                wt = pwq.tile([P, C], FP16, name=f"wkT{i}", tag=f"wkT{i}")
                nc.sync.dma_start(wt[:], d["wkT"][i * P:(i + 1) * P, :])
                wk_ch.append(wt)

            # PE warm-up during the DMA lead-in: matmuls on a zeroed tile
            # count as PE activity, so the HAM clock gate reaches K=8/8
            # before the first real matmul instead of ~3.4us into Q.
            pz = pps.tile([P, NT], F32, name="pz", tag="ps")
            for i in range(8):
                nc.tensor.matmul(pz[:], wz[:, 0:P], wz[:],
                                 start=(i == 0), stop=(i == 7))

            # persistent-pool inputs: emitted early so their DMAs queue
            # behind the q-stage inputs and stream during q compute
            c_ch = []
            for i in range(KC):
                t_ = p_c.tile([P, T], FP16, name=f"c{i}", tag=f"c{i}")
                nc.sync.dma_start(t_[:], d["ch"][i * P:(i + 1) * P, :])
                c_ch.append(t_)
            wv_ch = []
            for i in range(KC + 1):
                rows = slice(i * P, (i + 1) * P) if i < KC else slice(C, C + 1)
                t_ = p_wv.tile([P if i < KC else 1, VW], FP16,
                               name=f"wv{i}", tag=f"wv{i}")
                nc.sync.dma_start(t_[:], d["wvT"][rows, :])
                wv_ch.append(t_)
            wo_ch = []
            for i in range(KC):
                t_ = p_wo.tile([P, C], FP16, name=f"wo{i}", tag=f"wo{i}")
                nc.sync.dma_start(t_[:], d["woT"][i * P:(i + 1) * P, :])
                wo_ch.append(t_)
            bo4 = p_ms.tile([P, OC], F32, name="bo4", tag="bo4")
            nc.sync.dma_start(bo4[:], d["bo4"][:])

          # ---- Q projection + RoPE (projection, then rot = R q on the PE) --
          # The rot matmuls + RoPE tail for block m are emitted inside block
          # m+1's projection loop so the PE never waits on the psum->fp16
          # cast round-trip.
          if True:
            def flush_q_rot(m, qcs):
                for tb in range(2):
                    fs = slice(tb * NT, (tb + 1) * NT)
                    rp = pps.tile([P, NT], F32, name="rp", tag="ps")
                    nc.tensor.matmul(rp[:], rot_t[:], qcs[tb][:],
                                     start=True, stop=True)
                    t1 = px.tile([P, NT], FP16, name="rt1", tag="rt1", bufs=2)
                    nc.vector.tensor_mul(t1[:], qcs[tb][:], cosq[:, fs])
                    t2 = px.tile([P, NT], FP16, name="rt2", tag="rt2", bufs=2)
                    nc.vector.tensor_mul(t2[:], rp[:], sinq[:, fs])
                    nc.vector.tensor_add(q_sb[2 * m][0:DC, fs],
                                         t1[0:DC, :], t2[0:DC, :])
                    nc.vector.tensor_add(q_sb[2 * m + 1][DC:P, fs],
                                         t1[DC:P, :], t2[DC:P, :])

            carry_q = None
            for m in range(OC):
              ocs = slice(m * P, (m + 1) * P)
              psq = [pps.tile([P, NT], F32, name="psq", tag="ps")
                     for _ in range(2)]
              for kc in range(NK):
                st, spq = (kc == 0), (kc == NK - 1)
                for tb in range(2):  # same stationary, two t-blocks
                    fs = slice(tb * NT, (tb + 1) * NT)
                    nc.tensor.matmul(psq[tb][:], wq_ch[kc][:, ocs],
                                     x_ch[kc][:, fs], start=st, stop=spq)
                if kc == 1 and carry_q is not None:
                    flush_q_rot(*carry_q)
                    carry_q = None
              qcs = []
              for tb in range(2):
                qc = px.tile([P, NT], FP16, name="qc1", tag="qc1", bufs=4)
                nc.scalar.activation(qc[:], psq[tb][:], AF.Copy)
                qcs.append(qc)
              carry_q = (m, qcs)
            flush_q_rot(*carry_q)
          # ---- K projection + RoPE (projection, then rot = R k on the PE) --
          with tc.tile_pool(name="pwk", bufs=1) as pwk, \
               tc.tile_pool(name="ptk", bufs=1) as ptk:
            if use_bias:
                bk1 = pwk.tile([1, C], FP16, name="bk1", tag="bk1")
                nc.sync.dma_start(bk1[0:1, :], d["wkT"][C:C + 1, :])
                wk_ch = wk_ch + [bk1]
            cosk = ptk.tile([P, T], FP16, name="cosk", tag="cosk")
            nc.sync.dma_start(cosk[:], d["cosk"][:])
            sink = ptk.tile([P, T], FP16, name="sink", tag="sink")
            nc.sync.dma_start(sink[:], d["sink"][:])

            def flush_k_rot(oc, sbp, kcs):
                for i in range(2):
                    fs = slice((2 * sbp + i) * NT, (2 * sbp + i + 1) * NT)
                    rp = pps.tile([P, NT], F32, name="rpk", tag="ps")
                    nc.tensor.matmul(rp[:], rot_t[:], kcs[i][:],
                                     start=True, stop=True)
                    t1 = ptk.tile([P, NT], FP16, name="kt1", tag="kt1",
                                  bufs=2)
                    nc.vector.tensor_mul(t1[:], kcs[i][:], cosk[:, fs])
                    t2 = ptk.tile([P, NT], FP16, name="kt2", tag="kt2",
                                  bufs=2)
                    nc.vector.tensor_mul(t2[:], rp[:], sink[:, fs])
                    nc.vector.tensor_add(k_sb[oc][:, fs], t1[:], t2[:])

            carry_k = None
            for oc in range(OC):
                ocs = slice(oc * P, (oc + 1) * P)
                for sbp in range(2):  # s-block pairs
                    psk = [pps.tile([P, NT], F32, name="psk", tag="ps")
                           for _ in range(2)]
                    for kc in range(NK):
                        st, sp = (kc == 0), (kc == NK - 1)
                        cap = c_ch[kc] if kc < KC else ones1[0:1, :]
                        for i in range(2):
                            fs = slice((2 * sbp + i) * NT,
                                       (2 * sbp + i + 1) * NT)
                            nc.tensor.matmul(psk[i][:], wk_ch[kc][:, ocs],
                                             cap[:, fs], start=st, stop=sp)
                        if kc == 1 and carry_k is not None:
                            flush_k_rot(*carry_k)
                            carry_k = None
                    kcs = []
                    for i in range(2):
                        kcv = ptk.tile([P, NT], FP16, name="kc1", tag="kc1",
                                       bufs=4)
                        nc.scalar.activation(kcv[:], psk[i][:], AF.Copy)
                        kcs.append(kcv)
                    carry_k = (oc, sbp, kcs)
            flush_k_rot(*carry_k)

          # ---- V projection (transposed layout [s, o], 2x260-wide psum) ----
          # Shares the Q/K psum pool (a separate pool would add a scheduling
          # barrier at the K->V transition).  The wvT bias row (against the
          # all-ones stationary row) supplies the per-head ones columns (and
          # biases, when present).
          if True:
            for sc in range(SCN):
                scs = slice(sc * P, (sc + 1) * P)
                pv = [pps.tile([P, VH], F32, name=f"pv{i}", tag="ps")
                      for i in range(2)]
                for kc in range(KC + 1):
                    st, sp = (kc == 0), (kc == KC)
                    cap = (c_ch[kc][:, scs] if kc < KC
                           else ones1[0:1, scs])
                    for i in range(2):
                        cols = slice(i * VH, (i + 1) * VH)
                        nc.tensor.matmul(pv[i][:], cap,
                                         wv_ch[kc][:, cols], start=st, stop=sp)
                for i in range(2):
                    cols = slice(i * VH, (i + 1) * VH)
                    # on the scalar engine: the vector engine must be clear
                    # of V-phase work when the attention exp chains start
                    nc.scalar.activation(vt_sb[sc][:, cols], pv[i][:],
                                         AF.Copy)

        # ---- attention: scoresT -> exp -> AV (+denominator) -> normalize --
        with tc.tile_pool(name="ppsc", space="PSUM", bufs=2) as ppsc, \
             tc.tile_pool(name="ppav", space="PSUM", bufs=4) as ppav:
            # Denominator chains are software-pipelined one head back: head
            # h's reciprocal/broadcast/normalize is emitted in the middle of
            # head h+1's loop so its engine-FIFO slots never sit between the
            # PE's AV matmuls and the next head's score-psum reuse waits.
            def emit_den(av, m, rows):
                for tb in range(2):
                    fs = slice(tb * NT, (tb + 1) * NT)
                    # custom DVE ops misread PSUM rows at partition offset
                    # 64 — stage the denominator row to SBUF first
                    dr = pden.tile([1, NT], F32, name="dr", tag="dr",
                                   bufs=4)
                    nc.vector.tensor_copy(dr[0:1, :], av[tb][DC:DC + 1, :])
                    dn = pden.tile([1, NT], F32, name="dn", tag="dn",
                                   bufs=4)
                    nc.vector.reciprocal_approx_fast(dn[0:1, :], dr[0:1, :])
                    bcs = pden.tile([DC, NT], F32, name="bcs", tag="bcs",
                                    bufs=2)
                    nc.gpsimd.partition_broadcast(bcs[:], dn[0:1, :])
                    nc.vector.tensor_mul(att_sb[m][rows, fs],
                                         av[tb][0:DC, :], bcs[:])

            carry = None
            for m in range(OC):          # head pair (heads 2m, 2m+1)
                for hp in range(2):
                    h = 2 * m + hp
                    rows = slice(hp * DC, (hp + 1) * DC)
                    hs = slice(h * (DC + 1), h * (DC + 1) + P)
                    av = [ppav.tile([P, NT], F32, name="av", tag="av")
                          for _ in range(2)]
                    # AV accumulation into PSUM is order-independent, so AV
                    # matmuls are deferred: 1 chunk for ACT-exp'd tiles (hides
                    # ACT latency), to head-end for DVE-exp'd tiles (the DVE
                    # chain is ~2.4us; issuing its AV in sc order would stall
                    # the PE's in-order queue and oscillate the HAM clock).
                    pend = []   # (pt tile, sc) not yet fed to AV
                    dve_pend = []
                    n_issued = 0

                    def issue_av(pt, sc, first, last):
                        for tb in range(2):  # same vT stationary (128 wide)
                            nc.tensor.matmul(av[tb][:], vt_sb[sc][:, hs],
                                             pt[:, tb * NT:(tb + 1) * NT],
                                             start=first, stop=last)

                    for sc in range(SCN):
                        scs = slice(sc * P, (sc + 1) * P)
                        sp = ppsc.tile([P, 2 * NT], F32, name="sp", tag="sp")
                        for tb in range(2):  # same k-head stationary
                            fs = slice(tb * NT, (tb + 1) * NT)
                            nc.tensor.matmul(sp[:, tb * NT:(tb + 1) * NT],
                                             k_sb[m][:, scs], q_sb[h][:, fs],
                                             start=True, stop=True)
                        pt = ppt.tile([P, 2 * NT], FP16, name="pt", tag="pt",
                                      bufs=8)
                        if _is_dve_tile(h, sc):
                            p4 = pp48.tile([P, 2 * NT], FP16, name="p4",
                                           tag="p4", bufs=2)
                            nc.vector._custom_dve(
                                exp4, out=p4[:], in0=sp[:], in1=a0bc[:],
                                s0=EXPC[1], s1=EXPC[2], imm2=EXPC[3])
                            # single fused ^4 op keeps the vector FIFO short
                            # (exp4 completions gate score psum-slot reuse)
                            nc.vector._custom_dve(sqsq, out=pt[:], in0=p4[:])
                            dve_pend.append((pt, sc))
                        else:
                            nc.scalar.activation(pt[:], sp[:], AF.Exp,
                                                 scale=ESC)
                            pend.append((pt, sc))
                        # previous head's denominator work, mid-head
                        if sc == 3 and carry is not None:
                            emit_den(*carry)
                            carry = None
                        # flush one ACT tile, two sc behind (covers the
                        # ~1.1us ACT exp latency without stalling the PE)
                        if len(pend) >= 3:
                            fpt, fsc = pend.pop(0)
                            issue_av(fpt, fsc, n_issued == 0, False)
                            n_issued += 1
                    tail = pend + dve_pend
                    for i, (fpt, fsc) in enumerate(tail):
                        issue_av(fpt, fsc, n_issued == 0,
                                 i == len(tail) - 1)
                        n_issued += 1
                    carry = (av, m, rows)
            emit_den(*carry)

        # ---- output projection + bias ----
        with tc.tile_pool(name="ppy", space="PSUM", bufs=4) as ppy, \
             tc.tile_pool(name="pys", bufs=4) as pys:
            for of in range(OC):
                ofs = slice(of * P, (of + 1) * P)
                yp = [ppy.tile([P, NT], F32, name="yp", tag="yp")
                      for _ in range(2)]
                for oc in range(KC):
                    st, sp = (oc == 0), (oc == KC - 1)
                    for tb in range(2):  # same Wo stationary
                        fs = slice(tb * NT, (tb + 1) * NT)
                        nc.tensor.matmul(yp[tb][:], wo_ch[oc][:, ofs],
                                         att_sb[oc][:, fs], start=st, stop=sp)
                for tb in range(2):
                    fs = slice(tb * NT, (tb + 1) * NT)
                    ys = pys.tile([P, NT], FP16, name="ys", tag="ys")
                    nc.vector.tensor_scalar_add(ys[:], yp[tb][:],
                                                bo4[:, of:of + 1])
                    nc.sync.dma_start(y[ofs, fs], ys[:])


def build_program(use_bias=False):
    nc = bacc.Bacc("TRN2", target_bir_lowering=False, debug=False,
                   num_devices=NCORES)
    d = {}
    CE = C + 1 if use_bias else C

    def din(name, shape, dt=FP16):
        d[name] = nc.dram_tensor(name, shape, dt, kind="ExternalInput").ap()

    din("xh", [CE, TH])
    din("ch", [C, T])
    din("wqT", [CE, C])
    din("wkT", [CE, C])
    din("rotT", [P, P])
    din("wvT", [C + 1, VW])
    din("woT", [C, C])
    din("bo4", [P, OC], F32)
    din("cosq", [P, TH])
    din("sinq", [P, TH])
    din("cosk", [P, T])
    din("sink", [P, T])
    y = nc.dram_tensor("y", [C, TH], FP16, kind="ExternalOutput").ap()

    with tile.TileContext(nc) as tc:
        _emit(tc, d, y, use_bias)
    nc.compile()
    return nc


# --------------------------------------------------------------------------
# host-side input prep / output assembly
# --------------------------------------------------------------------------

def _rot_matrix():
    """R such that (R q)[i] matches reference rotate-half per 64-chan head."""
    R = np.zeros((C, C), np.float32)
    half = DC // 2
    for h in range(H):
        b0 = h * DC
        for i in range(half):
            R[b0 + i, b0 + half + i] = -1.0
            R[b0 + half + i, b0 + i] = 1.0
    return R


def _rope_tables():
    inv = 1.0 / (THETA ** (np.arange(0, DC, 2, dtype=np.float32) / DC))  # [32]
    f = np.arange(T, dtype=np.float32)[:, None] * inv[None, :]           # [T,32]
    pos = np.concatenate([f, f], axis=-1)                                # [T,64]
    cos_t, sin_t = np.cos(pos), np.sin(pos)                              # [T,64]
    # [128, T]: row r covers head-pair channel r, channel dim = r % 64
    cos_tab = np.ascontiguousarray(np.tile(cos_t.T, (2, 1)), np.float16)
    sin_tab = np.ascontiguousarray(np.tile(sin_t.T, (2, 1)), np.float16)
    return cos_tab, sin_tab


def make_in_maps(x, c, Wq, bq, Wk, bk, Wv, bv, Wo, bo, use_bias):
    scale = 1.0 / (math.sqrt(DC) * ESC)
    Wq_s, bq_s = Wq * scale, bq * scale

    def aug(Wt, bias):
        if not use_bias:
            return np.ascontiguousarray(Wt, np.float16)
        return np.ascontiguousarray(
            np.concatenate([Wt, bias[None, :]], axis=0), np.float16)

    wqT = aug(Wq_s.T, bq_s)
    wkT = aug(Wk.T, bk)
    # rotate-half matrix for a 2-head 128-row block, pre-transposed for
    # the stationary operand (rot = R q via lhsT = R.T)
    rotT = np.ascontiguousarray(_rot_matrix()[:P, :P].T, np.float16)

    # V^T always augmented: bias row carries per-head ones cols (h*65+64)
    wvT = np.zeros((C + 1, VW), np.float32)
    for h in range(H):
        wvT[:C, h * (DC + 1):h * (DC + 1) + DC] = Wv[h * DC:(h + 1) * DC, :].T
        wvT[C, h * (DC + 1):h * (DC + 1) + DC] = bv[h * DC:(h + 1) * DC]
        wvT[C, h * (DC + 1) + DC] = 1.0  # ones column via the bias row
    wvT = wvT.astype(np.float16)

    woT = np.ascontiguousarray(Wo.T, np.float16)
    bo4 = np.ascontiguousarray(bo.reshape(OC, P).T, np.float32)

    cos_tab, sin_tab = _rope_tables()

    ones_t = np.ones((1, TH), np.float16)
    in_maps = []
    for core in range(NCORES):
        b, j = core // 2, core % 2
        ts = slice(j * TH, (j + 1) * TH)
        if use_bias:
            xh = np.concatenate([x[b][:, ts].astype(np.float16), ones_t],
                                axis=0)
        else:
            xh = x[b][:, ts].astype(np.float16)
        in_maps.append({
            "xh": np.ascontiguousarray(xh, np.float16),
            "ch": np.ascontiguousarray(c[b], np.float16),
            "wqT": wqT, "wkT": wkT, "rotT": rotT,
            "wvT": wvT, "woT": woT, "bo4": bo4,
            "cosq": np.ascontiguousarray(cos_tab[:, ts]),
            "sinq": np.ascontiguousarray(sin_tab[:, ts]),
            "cosk": cos_tab,
            "sink": sin_tab,
        })
    return in_maps


def assemble(results):
    Y = np.empty((B, C, T), np.float32)
    for core in range(NCORES):
        b, j = core // 2, core % 2
        Y[b, :, j * TH:(j + 1) * TH] = results[core]["y"]
    return Y


_CACHE = {}


def _get_program(use_bias):
    if use_bias not in _CACHE:
        _CACHE[use_bias] = build_program(use_bias)
    return _CACHE[use_bias]


def run(trace=False, **inputs):
    use_bias = any(
        np.any(np.asarray(inputs[k])) for k in ("bq", "bk", "bv"))
    nc = _get_program(use_bias)
    in_maps = make_in_maps(use_bias=use_bias, **inputs)
    res = bass_utils.run_bass_kernel_spmd(
        nc, in_maps, core_ids=list(range(NCORES)), trace=trace)
    return assemble(res.results), res


def kernel(**inputs):
    out, _ = run(trace=False, **inputs)
    return out
